# revision 21
# baseline (speedup 1.0000x reference)
"""LocalConv2D (3x3, width split into 4 weight blocks, 4-bit fake-quant weights)
on 8 Trainium2 NeuronCores.

Strategy
--------
Data-parallel over batch: 32 images -> 4 per core, processed as 2 pairs.
Image A of a pair lives in SBUF partitions 0-63 (its 64 channels), image B
in partitions 64-127. The 3x3 conv is 9 shifted K=64 matmuls accumulated in
PSUM; A's matmuls run in PE row-group 0 and B's in row-group 64
(tile_position auto-derived), alternating so both halves of the 128x128
array stream near-concurrently. This K=64 structure is the one that keeps
the PE HAM un-throttled at 2.4 GHz on this part (full-height K=128 streams
measured stuck at the 1.2 GHz cold clock for entire kernels).

Perf structure:
- PE warmup: dense dummy matmul groups alternating two PSUM banks start at
  t=0 (no DMA dependency) so the HAM clock-gate releases before real work.
- Input strips split into overlapping h-halves (30 rows each) so the first
  matmul only waits for a 445KB DMA, not the full image pair.
- Weights DMA'd in per-b chunks on the Activation HWDGE ring while input
  strips use the SP ring -> the two DMA queues pipeline concurrently.
- Output staged in bf16 (halves DMA traffic; fro-error ~2.3e-3 stays well
  under the 2e-2 gate) and streamed out per h-half.

Weights are fake-quantized per-tensor to 4 bits: q = round(w/s)*s with
s = max|w|/7. round(w/s) is a small integer in [-7,7], exactly
representable in bf16, so the matmul runs on exact integer weights and the
scale s is folded into the eviction (out = psum*s + bias).
"""

import numpy as np

KSIZE = 3
SW = 4
KBITS = 4
N, C, H, W, F = 32, 64, 56, 56, 128
HP, WP = H + 2, W + 2          # padded 58x58
N_CORES = 8
IMGS_PER_CORE = N // N_CORES   # 4
PAIRS = IMGS_PER_CORE // 2     # 2
WB = W // SW                   # 14
HH = H // 2                    # 28 rows per h-half tile (28*14 = 392 <= 512)
HR = HH + 2                    # 30 input rows feeding one h-half

_COMPILED = {}


def _install_drain_patch():
    """The walrus build here rejects instructions carrying >2 sync waits
    ('Too many sync wait commands'). Two fixes, both relying on engines
    executing their own stream in order:

    1. _add_instruction: any scheduled instruction with >2 waits gets
       same-engine NoOps inserted before it, each carrying <=2 of the waits.
    2. The Tile tail drain gets one wait per outstanding logical proc; emit
       one SP nop per proc, then strip the duplicated waits off the drain.
    """
    import re
    import bass_rust
    from concourse.vector_clock import ScopedClock
    import concourse.tile as tile
    import concourse.mybir as mybir

    if getattr(tile.TileContext, "_drain_patch_installed", False):
        return

    MAXW = 1       # this walrus build fits exactly 1 sync wait per instruction
    NOP_MAXW = 1
    _orig_add = tile.TileContext._add_instruction

    def _add_split(self, inst):
        si = getattr(inst, "sync_info", None)
        if si is not None and si.on_wait and len(si.on_wait) > MAXW:
            waits = list(si.on_wait)
            while len(waits) > MAXW:
                chunk, waits = waits[:NOP_MAXW], waits[NOP_MAXW:]
                nop = mybir.InstNoOp(
                    name=self.nc.get_next_instruction_name(), ins=[], outs=[]
                )
                nop.engine = inst.engine
                nop.sync_info = bass_rust.SyncInfo(on_wait=chunk, on_update=[])
                _orig_add(self, nop)
            si.on_wait = waits
        return _orig_add(self, inst)

    tile.TileContext._add_instruction = _add_split

    _orig = tile.TileContext._drain_and_barrier

    def _split(self, tick_clock, wait_clock):
        gc = tick_clock.global_clock
        m = re.match(r"VectorClock\(\[(.*)\]\)", repr(gc))
        vals = [int(v) for v in m.group(1).split(",")] if m.group(1).strip() else []
        covered = set()
        # Round-robin the per-proc wait nops across engines so they wait in
        # parallel (serial on one engine costs ~55ns each x ~30 procs). The
        # all-engine barrier that _orig emits right after collects them.
        engines = [
            self.nc.sync, self.nc.scalar, self.nc.vector,
            self.nc.gpsimd, self.nc.tensor,
        ]
        eng_i = 0
        for i, v in enumerate(vals):
            if v == 0:
                continue
            part = [0] * len(vals)
            part[i] = v
            nop = engines[eng_i % len(engines)].nop()
            eng_i += 1
            wait_clock.add_sem_waits(
                nop.ins, ScopedClock({None: bass_rust.VectorClock(part)})
            )
            si = nop.ins.sync_info
            for w in (si.on_wait if si else []) or []:
                covered.add((w.ant_name, w.wait_value))
        holder = []
        orig_drain = self.nc.sync.drain

        def capture(*a, **k):
            inst = orig_drain(*a, **k)
            holder.append(inst)
            return inst

        self.nc.sync.drain = capture
        try:
            r = _orig(self, tick_clock, wait_clock)
        finally:
            self.nc.sync.drain = orig_drain
        if holder:
            inst = holder[0].ins if hasattr(holder[0], "ins") else holder[0]
            si = inst.sync_info
            if si and si.on_wait:
                si.on_wait = [
                    w for w in si.on_wait
                    if (w.ant_name, w.wait_value) not in covered
                ]
        return r

    tile.TileContext._drain_and_barrier = _split
    tile.TileContext._drain_patch_installed = True


def _build_program():
    import concourse.bass as bass
    import concourse.mybir as mybir
    import concourse.tile as tile

    _install_drain_patch()

    f32 = mybir.dt.float32
    bf16 = mybir.dt.bfloat16

    nc = bass.Bass(target_bir_lowering=False, debug=False)
    # per-core inputs: image pairs packed [128, HP*WP] (A in 0-63, B 64-127)
    xin = nc.declare_dram_parameter(
        "xin", [PAIRS, 128, HP * WP], bf16, isOutput=False
    )
    # weights duplicated across halves: col (b*9 + t)*F + f
    wts = nc.declare_dram_parameter("wts", [128, SW * 9 * F], bf16, isOutput=False)
    bsc_p = nc.declare_dram_parameter("bsc", [128, 2], f32, isOutput=False)
    # b-major output layout: block (b, hh) of an image is contiguous, so
    # each PSUM eviction can stream straight to DRAM. Host reassembles.
    y = nc.declare_dram_parameter(
        "y", [IMGS_PER_CORE, 128, SW, H * WB], bf16, isOutput=True
    )

    WCH = 9 * F                  # weight columns per b-block

    N_WGRP = 3                   # warmup groups of 4 N=256 matmuls each

    with tile.TileContext(nc) as tc:
        with (
            tc.tile_pool(name="consts", bufs=1) as consts,
            tc.tile_pool(name="strips", bufs=4) as strips,
            tc.tile_pool(name="stage", bufs=3) as stage,
            tc.tile_pool(name="psum", bufs=3, space="PSUM") as psum,
            tc.tile_pool(name="wpsum", bufs=2, space="PSUM") as wpsum,
        ):
            # PE warmup with no DMA dependency: accumulation groups of 4
            # back-to-back matmuls, alternating between two PSUM banks so
            # one bank drains while the other streams -> near-100% PE duty,
            # which is what the HAM busy-detector needs to un-throttle.
            warm_src = consts.tile([64, 256], bf16)
            nc.vector.memset(warm_src[:], 0.0)
            for _ in range(N_WGRP):
                wps = wpsum.tile([64, 256], f32, tag="warm")
                for k in range(4):
                    nc.tensor.matmul(
                        wps[:], warm_src[0:64, 0:64], warm_src[0:64, 0:256],
                        start=(k == 0), stop=(k == 3),
                    )

            wt = consts.tile([128, SW * 9 * F], bf16)
            bsc_t = consts.tile([128, 2], f32)

            # DMA issue order per ring is FIFO: first strip split across
            # both rings by partition half (complementary DMA-engine sets ->
            # full aggregate bandwidth), then b0 weights + bias, then the
            # rest; strips on SP ring, weights on ACT ring -> both pipeline.
            halves = {}
            sh0 = strips.tile([128, HR * WP], bf16, tag="strip")
            src0 = xin[0, :, 0:HR * WP]
            nc.sync.dma_start(sh0[0:64, :], src0[0:64, :])
            nc.scalar.dma_start(sh0[64:128, :], src0[64:128, :])
            halves[(0, 0)] = sh0.rearrange("p (h w) -> p h w", w=WP)

            nc.scalar.dma_start(wt[:, 0:WCH], wts[:, 0:WCH])
            nc.scalar.dma_start(bsc_t[:], bsc_p[:])
            for b in range(1, SW):
                nc.scalar.dma_start(
                    wt[:, b * WCH:(b + 1) * WCH], wts[:, b * WCH:(b + 1) * WCH]
                )

            for q in range(PAIRS):
                for hh in range(2):
                    if (q, hh) in halves:
                        continue
                    sh = strips.tile([128, HR * WP], bf16, tag="strip")
                    nc.sync.dma_start(
                        sh[:], xin[q, :, hh * HH * WP:(hh * HH + HR) * WP]
                    )
                    halves[(q, hh)] = sh.rearrange("p (h w) -> p h w", w=WP)

            for q in range(PAIRS):
                for hh in range(2):
                    sqv = halves[(q, hh)]
                    for b in range(SW):
                        ps_a = psum.tile([128, HH * WB], f32, tag="ps_a")
                        ps_b = psum.tile([128, HH * WB], f32, tag="ps_b")
                        for t in range(9):
                            i, j = t // 3, t % 3
                            first, last = t == 0, t == 8
                            wcol = (b * 9 + t) * F
                            rhs_a = sqv[0:64, i:i + HH,
                                        b * WB + j:b * WB + j + WB]
                            rhs_b = sqv[64:128, i:i + HH,
                                        b * WB + j:b * WB + j + WB]
                            nc.tensor.matmul(
                                ps_a[:], wt[0:64, wcol:wcol + F], rhs_a,
                                start=first, stop=last,
                            )
                            nc.tensor.matmul(
                                ps_b[:], wt[64:128, wcol:wcol + F], rhs_b,
                                start=first, stop=last,
                            )
                        # evict with fused quant-scale + bias into contiguous
                        # bf16 block tiles, then stream each straight out
                        ev_a = stage.tile([128, HH * WB], bf16, tag="ev_a")
                        ev_b = stage.tile([128, HH * WB], bf16, tag="ev_b")
                        nc.scalar.activation(
                            ev_a[:], ps_a[:],
                            mybir.ActivationFunctionType.Identity,
                            bias=bsc_t[:, 0:1], scale=bsc_t[:, 1:2],
                        )
                        nc.vector.tensor_scalar(
                            ev_b[:], ps_b[:], bsc_t[:, 1:2], bsc_t[:, 0:1],
                            mybir.AluOpType.mult, mybir.AluOpType.add,
                        )
                        lo, hi = hh * HH * WB, (hh + 1) * HH * WB
                        nc.sync.dma_start(y[2 * q, :, b, lo:hi], ev_a[:])
                        nc.scalar.dma_start(y[2 * q + 1, :, b, lo:hi], ev_b[:])

    nc.finalize()
    return nc


def _prep_inputs(inputs, kernel, bias):
    import ml_dtypes

    # fake-quant: integer part exact in bf16, scale folded into eviction
    scale = float(np.max(np.abs(kernel)) / 7.0)
    w_int = np.round(kernel[0] / scale).astype(np.float32)  # [SW, 576, F]

    # weight layout: [128 partitions, SW*9*F]; partition p holds channel
    # c = p % 64 (duplicated across halves for the two PE row groups)
    # free index (b*9 + t)*F + f  ->  w_int[b, c*9 + t, f]
    w4 = w_int.reshape(SW, C, 9, F)                    # [b, c, t, f]
    wt_half = np.transpose(w4, (1, 0, 2, 3)).reshape(C, SW * 9 * F)
    wts_host = np.concatenate([wt_half, wt_half], axis=0)  # [128, SW*9*F]
    wts_host = wts_host.astype(ml_dtypes.bfloat16)

    # input: pad spatially, bf16, pack image pairs into 128 partitions
    xp = np.zeros((N, C, HP, WP), np.float32)
    xp[:, :, 1:-1, 1:-1] = inputs
    xp = xp.astype(ml_dtypes.bfloat16).reshape(N, C, HP * WP)

    bsc_host = np.zeros((128, 2), np.float32)
    bsc_host[:, 0] = np.ascontiguousarray(bias, np.float32)
    bsc_host[:, 1] = scale

    in_maps = []
    for core in range(N_CORES):
        base = core * IMGS_PER_CORE
        xin = np.empty((PAIRS, 128, HP * WP), ml_dtypes.bfloat16)
        for q in range(PAIRS):
            xin[q, 0:64] = xp[base + 2 * q]
            xin[q, 64:128] = xp[base + 2 * q + 1]
        in_maps.append({
            "xin": xin,
            "wts": wts_host,
            "bsc": bsc_host,
        })
    return in_maps


def kernel(inputs, kernel, bias, _trace=False):
    from concourse.bass_utils import run_bass_kernel_spmd

    inputs = np.asarray(inputs)
    kernel = np.asarray(kernel)
    bias = np.asarray(bias)

    if "nc" not in _COMPILED:
        _COMPILED["nc"] = _build_program()
    nc = _COMPILED["nc"]

    in_maps = _prep_inputs(inputs, kernel, bias)
    res = run_bass_kernel_spmd(
        nc, in_maps, list(range(N_CORES)), trace=_trace
    )
    out = np.empty((N, F, H, W), np.float32)
    for core in range(N_CORES):
        base = core * IMGS_PER_CORE
        # y is b-major: [img, f, b, h*WB+wb] -> [img, f, h, b*WB+wb]
        yc = res.results[core]["y"].astype(np.float32)
        yc = yc.reshape(IMGS_PER_CORE, F, SW, H, WB).transpose(0, 1, 3, 2, 4)
        out[base:base + IMGS_PER_CORE] = yc.reshape(IMGS_PER_CORE, F, H, W)
    if _trace:
        return out, res
    return out


# revision 22
# speedup vs baseline: 1.0094x; 1.0094x over previous
"""LocalConv2D (3x3, width split into 4 weight blocks, 4-bit fake-quant weights)
on 8 Trainium2 NeuronCores.

Strategy
--------
Data-parallel over batch: 32 images -> 4 per core, processed as 2 pairs.
Image A of a pair lives in SBUF partitions 0-63 (its 64 channels), image B
in partitions 64-127. The 3x3 conv is 9 shifted K=64 matmuls accumulated in
PSUM; A's matmuls run in PE row-group 0 and B's in row-group 64
(tile_position auto-derived), alternating so both halves of the 128x128
array stream near-concurrently. This K=64 structure is the one that keeps
the PE HAM un-throttled at 2.4 GHz on this part (full-height K=128 streams
measured stuck at the 1.2 GHz cold clock for entire kernels).

Perf structure:
- PE warmup: dense dummy matmul groups alternating two PSUM banks start at
  t=0 (no DMA dependency) so the HAM clock-gate releases before real work.
- Input strips split into overlapping h-halves (30 rows each) so the first
  matmul only waits for a 445KB DMA, not the full image pair.
- Weights DMA'd in per-b chunks on the Activation HWDGE ring while input
  strips use the SP ring -> the two DMA queues pipeline concurrently.
- Output staged in bf16 (halves DMA traffic; fro-error ~2.3e-3 stays well
  under the 2e-2 gate) and streamed out per h-half.

Weights are fake-quantized per-tensor to 4 bits: q = round(w/s)*s with
s = max|w|/7. round(w/s) is a small integer in [-7,7], exactly
representable in bf16, so the matmul runs on exact integer weights and the
scale s is folded into the eviction (out = psum*s + bias).
"""

import numpy as np

KSIZE = 3
SW = 4
KBITS = 4
N, C, H, W, F = 32, 64, 56, 56, 128
HP, WP = H + 2, W + 2          # padded 58x58
N_CORES = 8
IMGS_PER_CORE = N // N_CORES   # 4
PAIRS = IMGS_PER_CORE // 2     # 2
WB = W // SW                   # 14
HH = H // 2                    # 28 rows per h-half tile (28*14 = 392 <= 512)
HR = HH + 2                    # 30 input rows feeding one h-half

_COMPILED = {}


def _install_drain_patch():
    """The walrus build here rejects instructions carrying >2 sync waits
    ('Too many sync wait commands'). Two fixes, both relying on engines
    executing their own stream in order:

    1. _add_instruction: any scheduled instruction with >2 waits gets
       same-engine NoOps inserted before it, each carrying <=2 of the waits.
    2. The Tile tail drain gets one wait per outstanding logical proc; emit
       one SP nop per proc, then strip the duplicated waits off the drain.
    """
    import re
    import bass_rust
    from concourse.vector_clock import ScopedClock
    import concourse.tile as tile
    import concourse.mybir as mybir

    if getattr(tile.TileContext, "_drain_patch_installed", False):
        return

    MAXW = 1       # this walrus build fits exactly 1 sync wait per instruction
    NOP_MAXW = 1
    _orig_add = tile.TileContext._add_instruction

    def _add_split(self, inst):
        si = getattr(inst, "sync_info", None)
        if si is not None and si.on_wait and len(si.on_wait) > MAXW:
            waits = list(si.on_wait)
            while len(waits) > MAXW:
                chunk, waits = waits[:NOP_MAXW], waits[NOP_MAXW:]
                nop = mybir.InstNoOp(
                    name=self.nc.get_next_instruction_name(), ins=[], outs=[]
                )
                nop.engine = inst.engine
                nop.sync_info = bass_rust.SyncInfo(on_wait=chunk, on_update=[])
                _orig_add(self, nop)
            si.on_wait = waits
        return _orig_add(self, inst)

    tile.TileContext._add_instruction = _add_split

    _orig = tile.TileContext._drain_and_barrier

    def _split(self, tick_clock, wait_clock):
        gc = tick_clock.global_clock
        m = re.match(r"VectorClock\(\[(.*)\]\)", repr(gc))
        vals = [int(v) for v in m.group(1).split(",")] if m.group(1).strip() else []
        covered = set()
        # Round-robin the per-proc wait nops across engines so they wait in
        # parallel (serial on one engine costs ~55ns each x ~30 procs). The
        # all-engine barrier that _orig emits right after collects them.
        engines = [
            self.nc.sync, self.nc.scalar, self.nc.vector,
            self.nc.gpsimd, self.nc.tensor,
        ]
        eng_i = 0
        for i, v in enumerate(vals):
            if v == 0:
                continue
            part = [0] * len(vals)
            part[i] = v
            nop = engines[eng_i % len(engines)].nop()
            eng_i += 1
            wait_clock.add_sem_waits(
                nop.ins, ScopedClock({None: bass_rust.VectorClock(part)})
            )
            si = nop.ins.sync_info
            for w in (si.on_wait if si else []) or []:
                covered.add((w.ant_name, w.wait_value))
        holder = []
        orig_drain = self.nc.sync.drain

        def capture(*a, **k):
            inst = orig_drain(*a, **k)
            holder.append(inst)
            return inst

        self.nc.sync.drain = capture
        try:
            r = _orig(self, tick_clock, wait_clock)
        finally:
            self.nc.sync.drain = orig_drain
        if holder:
            inst = holder[0].ins if hasattr(holder[0], "ins") else holder[0]
            si = inst.sync_info
            if si and si.on_wait:
                si.on_wait = [
                    w for w in si.on_wait
                    if (w.ant_name, w.wait_value) not in covered
                ]
        return r

    tile.TileContext._drain_and_barrier = _split
    tile.TileContext._drain_patch_installed = True


def _build_program():
    import concourse.bass as bass
    import concourse.mybir as mybir
    import concourse.tile as tile

    _install_drain_patch()

    f32 = mybir.dt.float32
    bf16 = mybir.dt.bfloat16

    nc = bass.Bass(target_bir_lowering=False, debug=False)
    # per-core inputs: image pairs packed [128, HP*WP] (A in 0-63, B 64-127)
    xin = nc.declare_dram_parameter(
        "xin", [PAIRS, 128, HP * WP], bf16, isOutput=False
    )
    # weights duplicated across halves: col (b*9 + t)*F + f
    wts = nc.declare_dram_parameter("wts", [128, SW * 9 * F], bf16, isOutput=False)
    bsc_p = nc.declare_dram_parameter("bsc", [128, 2], f32, isOutput=False)
    # b-major output layout: block (b, hh) of an image is contiguous, so
    # each PSUM eviction can stream straight to DRAM. Host reassembles.
    y = nc.declare_dram_parameter(
        "y", [IMGS_PER_CORE, 128, SW, H * WB], bf16, isOutput=True
    )

    WCH = 9 * F                  # weight columns per b-block

    N_WGRP = 6                   # warmup groups of 4 N=256 matmuls each

    with tile.TileContext(nc) as tc:
        with (
            tc.tile_pool(name="consts", bufs=1) as consts,
            tc.tile_pool(name="strips", bufs=4) as strips,
            tc.tile_pool(name="stage", bufs=3) as stage,
            tc.tile_pool(name="psum", bufs=3, space="PSUM") as psum,
            tc.tile_pool(name="wpsum", bufs=2, space="PSUM") as wpsum,
        ):
            # PE warmup with no DMA dependency: accumulation groups of 4
            # back-to-back matmuls, alternating between two PSUM banks so
            # one bank drains while the other streams -> near-100% PE duty,
            # which is what the HAM busy-detector needs to un-throttle.
            warm_src = consts.tile([64, 256], bf16)
            nc.vector.memset(warm_src[:], 0.0)
            for _ in range(N_WGRP):
                wps = wpsum.tile([64, 256], f32, tag="warm")
                for k in range(4):
                    nc.tensor.matmul(
                        wps[:], warm_src[0:64, 0:64], warm_src[0:64, 0:256],
                        start=(k == 0), stop=(k == 3),
                    )

            wt = consts.tile([128, SW * 9 * F], bf16)
            bsc_t = consts.tile([128, 2], f32)
            # weights on the ACT HWDGE ring (per-b chunks), strips on SP
            # ring -> the two rings pipeline; b0 weights + bias first so the
            # PE and the first eviction are unblocked earliest
            nc.scalar.dma_start(wt[:, 0:WCH], wts[:, 0:WCH])
            nc.scalar.dma_start(bsc_t[:], bsc_p[:])
            for b in range(1, SW):
                nc.scalar.dma_start(
                    wt[:, b * WCH:(b + 1) * WCH], wts[:, b * WCH:(b + 1) * WCH]
                )

            halves = {}
            for q in range(PAIRS):
                for hh in range(2):
                    sh = strips.tile([128, HR * WP], bf16, tag="strip")
                    nc.sync.dma_start(
                        sh[:], xin[q, :, hh * HH * WP:(hh * HH + HR) * WP]
                    )
                    halves[(q, hh)] = sh.rearrange("p (h w) -> p h w", w=WP)

            for q in range(PAIRS):
                for hh in range(2):
                    sqv = halves[(q, hh)]
                    for b in range(SW):
                        ps_a = psum.tile([128, HH * WB], f32, tag="ps_a")
                        ps_b = psum.tile([128, HH * WB], f32, tag="ps_b")
                        for t in range(9):
                            i, j = t // 3, t % 3
                            first, last = t == 0, t == 8
                            wcol = (b * 9 + t) * F
                            rhs_a = sqv[0:64, i:i + HH,
                                        b * WB + j:b * WB + j + WB]
                            rhs_b = sqv[64:128, i:i + HH,
                                        b * WB + j:b * WB + j + WB]
                            nc.tensor.matmul(
                                ps_a[:], wt[0:64, wcol:wcol + F], rhs_a,
                                start=first, stop=last,
                            )
                            nc.tensor.matmul(
                                ps_b[:], wt[64:128, wcol:wcol + F], rhs_b,
                                start=first, stop=last,
                            )
                        # evict with fused quant-scale + bias into contiguous
                        # bf16 block tiles, then stream each straight out
                        ev_a = stage.tile([128, HH * WB], bf16, tag="ev_a")
                        ev_b = stage.tile([128, HH * WB], bf16, tag="ev_b")
                        nc.scalar.activation(
                            ev_a[:], ps_a[:],
                            mybir.ActivationFunctionType.Identity,
                            bias=bsc_t[:, 0:1], scale=bsc_t[:, 1:2],
                        )
                        nc.vector.tensor_scalar(
                            ev_b[:], ps_b[:], bsc_t[:, 1:2], bsc_t[:, 0:1],
                            mybir.AluOpType.mult, mybir.AluOpType.add,
                        )
                        lo, hi = hh * HH * WB, (hh + 1) * HH * WB
                        nc.sync.dma_start(y[2 * q, :, b, lo:hi], ev_a[:])
                        nc.scalar.dma_start(y[2 * q + 1, :, b, lo:hi], ev_b[:])

    nc.finalize()
    return nc


def _prep_inputs(inputs, kernel, bias):
    import ml_dtypes

    # fake-quant: integer part exact in bf16, scale folded into eviction
    scale = float(np.max(np.abs(kernel)) / 7.0)
    w_int = np.round(kernel[0] / scale).astype(np.float32)  # [SW, 576, F]

    # weight layout: [128 partitions, SW*9*F]; partition p holds channel
    # c = p % 64 (duplicated across halves for the two PE row groups)
    # free index (b*9 + t)*F + f  ->  w_int[b, c*9 + t, f]
    w4 = w_int.reshape(SW, C, 9, F)                    # [b, c, t, f]
    wt_half = np.transpose(w4, (1, 0, 2, 3)).reshape(C, SW * 9 * F)
    wts_host = np.concatenate([wt_half, wt_half], axis=0)  # [128, SW*9*F]
    wts_host = wts_host.astype(ml_dtypes.bfloat16)

    # input: pad spatially, bf16, pack image pairs into 128 partitions
    xp = np.zeros((N, C, HP, WP), np.float32)
    xp[:, :, 1:-1, 1:-1] = inputs
    xp = xp.astype(ml_dtypes.bfloat16).reshape(N, C, HP * WP)

    bsc_host = np.zeros((128, 2), np.float32)
    bsc_host[:, 0] = np.ascontiguousarray(bias, np.float32)
    bsc_host[:, 1] = scale

    in_maps = []
    for core in range(N_CORES):
        base = core * IMGS_PER_CORE
        xin = np.empty((PAIRS, 128, HP * WP), ml_dtypes.bfloat16)
        for q in range(PAIRS):
            xin[q, 0:64] = xp[base + 2 * q]
            xin[q, 64:128] = xp[base + 2 * q + 1]
        in_maps.append({
            "xin": xin,
            "wts": wts_host,
            "bsc": bsc_host,
        })
    return in_maps


def kernel(inputs, kernel, bias, _trace=False):
    from concourse.bass_utils import run_bass_kernel_spmd

    inputs = np.asarray(inputs)
    kernel = np.asarray(kernel)
    bias = np.asarray(bias)

    if "nc" not in _COMPILED:
        _COMPILED["nc"] = _build_program()
    nc = _COMPILED["nc"]

    in_maps = _prep_inputs(inputs, kernel, bias)
    res = run_bass_kernel_spmd(
        nc, in_maps, list(range(N_CORES)), trace=_trace
    )
    out = np.empty((N, F, H, W), np.float32)
    for core in range(N_CORES):
        base = core * IMGS_PER_CORE
        # y is b-major: [img, f, b, h*WB+wb] -> [img, f, h, b*WB+wb]
        yc = res.results[core]["y"].astype(np.float32)
        yc = yc.reshape(IMGS_PER_CORE, F, SW, H, WB).transpose(0, 1, 3, 2, 4)
        out[base:base + IMGS_PER_CORE] = yc.reshape(IMGS_PER_CORE, F, H, W)
    if _trace:
        return out, res
    return out


# revision 25
# speedup vs baseline: 1.0235x; 1.0140x over previous
"""LocalConv2D (3x3, width split into 4 weight blocks, 4-bit fake-quant weights)
on 8 Trainium2 NeuronCores.

Strategy
--------
Data-parallel over batch: 32 images -> 4 per core, processed as 2 pairs.
Image A of a pair lives in SBUF partitions 0-63 (its 64 channels), image B
in partitions 64-127. The 3x3 conv is 9 shifted K=64 matmuls accumulated in
PSUM; A's matmuls run in PE row-group 0 and B's in row-group 64
(tile_position auto-derived), alternating so both halves of the 128x128
array stream near-concurrently. This K=64 structure is the one that keeps
the PE HAM un-throttled at 2.4 GHz on this part (full-height K=128 streams
measured stuck at the 1.2 GHz cold clock for entire kernels).

Perf structure:
- PE warmup: dense dummy matmul groups alternating two PSUM banks start at
  t=0 (no DMA dependency) so the HAM clock-gate releases before real work.
- Input strips split into overlapping h-halves (30 rows each) so the first
  matmul only waits for a 445KB DMA, not the full image pair.
- Weights DMA'd in per-b chunks on the Activation HWDGE ring while input
  strips use the SP ring -> the two DMA queues pipeline concurrently.
- Output staged in bf16 (halves DMA traffic; fro-error ~2.3e-3 stays well
  under the 2e-2 gate) and streamed out per h-half.

Weights are fake-quantized per-tensor to 4 bits: q = round(w/s)*s with
s = max|w|/7. round(w/s) is a small integer in [-7,7], exactly
representable in bf16, so the matmul runs on exact integer weights and the
scale s is folded into the eviction (out = psum*s + bias).
"""

import numpy as np

KSIZE = 3
SW = 4
KBITS = 4
N, C, H, W, F = 32, 64, 56, 56, 128
HP, WP = H + 2, W + 2          # padded 58x58
N_CORES = 8
IMGS_PER_CORE = N // N_CORES   # 4
PAIRS = IMGS_PER_CORE // 2     # 2
WB = W // SW                   # 14
HH = H // 2                    # 28 rows per h-half tile (28*14 = 392 <= 512)
HR = HH + 2                    # 30 input rows feeding one h-half

_COMPILED = {}


def _install_drain_patch():
    """The walrus build here rejects instructions carrying >2 sync waits
    ('Too many sync wait commands'). Two fixes, both relying on engines
    executing their own stream in order:

    1. _add_instruction: any scheduled instruction with >2 waits gets
       same-engine NoOps inserted before it, each carrying <=2 of the waits.
    2. The Tile tail drain gets one wait per outstanding logical proc; emit
       one SP nop per proc, then strip the duplicated waits off the drain.
    """
    import re
    import bass_rust
    from concourse.vector_clock import ScopedClock
    import concourse.tile as tile
    import concourse.mybir as mybir

    if getattr(tile.TileContext, "_drain_patch_installed", False):
        return

    MAXW = 1       # this walrus build fits exactly 1 sync wait per instruction
    NOP_MAXW = 1
    _orig_add = tile.TileContext._add_instruction

    def _add_split(self, inst):
        si = getattr(inst, "sync_info", None)
        if si is not None and si.on_wait and len(si.on_wait) > MAXW:
            waits = list(si.on_wait)
            while len(waits) > MAXW:
                chunk, waits = waits[:NOP_MAXW], waits[NOP_MAXW:]
                nop = mybir.InstNoOp(
                    name=self.nc.get_next_instruction_name(), ins=[], outs=[]
                )
                nop.engine = inst.engine
                nop.sync_info = bass_rust.SyncInfo(on_wait=chunk, on_update=[])
                _orig_add(self, nop)
            si.on_wait = waits
        return _orig_add(self, inst)

    tile.TileContext._add_instruction = _add_split

    _orig = tile.TileContext._drain_and_barrier

    def _split(self, tick_clock, wait_clock):
        gc = tick_clock.global_clock
        m = re.match(r"VectorClock\(\[(.*)\]\)", repr(gc))
        vals = [int(v) for v in m.group(1).split(",")] if m.group(1).strip() else []
        covered = set()
        # Round-robin the per-proc wait nops across engines so they wait in
        # parallel (serial on one engine costs ~55ns each x ~30 procs). The
        # all-engine barrier that _orig emits right after collects them.
        engines = [
            self.nc.sync, self.nc.scalar, self.nc.vector,
            self.nc.gpsimd, self.nc.tensor,
        ]
        eng_i = 0
        for i, v in enumerate(vals):
            if v == 0:
                continue
            part = [0] * len(vals)
            part[i] = v
            nop = engines[eng_i % len(engines)].nop()
            eng_i += 1
            wait_clock.add_sem_waits(
                nop.ins, ScopedClock({None: bass_rust.VectorClock(part)})
            )
            si = nop.ins.sync_info
            for w in (si.on_wait if si else []) or []:
                covered.add((w.ant_name, w.wait_value))
        holder = []
        orig_drain = self.nc.sync.drain

        def capture(*a, **k):
            inst = orig_drain(*a, **k)
            holder.append(inst)
            return inst

        self.nc.sync.drain = capture
        try:
            r = _orig(self, tick_clock, wait_clock)
        finally:
            self.nc.sync.drain = orig_drain
        if holder:
            inst = holder[0].ins if hasattr(holder[0], "ins") else holder[0]
            si = inst.sync_info
            if si and si.on_wait:
                si.on_wait = [
                    w for w in si.on_wait
                    if (w.ant_name, w.wait_value) not in covered
                ]
        return r

    tile.TileContext._drain_and_barrier = _split
    tile.TileContext._drain_patch_installed = True


def _build_program():
    import concourse.bass as bass
    import concourse.mybir as mybir
    import concourse.tile as tile

    _install_drain_patch()

    f32 = mybir.dt.float32
    bf16 = mybir.dt.bfloat16

    nc = bass.Bass(target_bir_lowering=False, debug=False)
    # per-core inputs: image pairs packed [128, HP*WP] (A in 0-63, B 64-127)
    xin = nc.declare_dram_parameter(
        "xin", [PAIRS, 128, HP * WP], bf16, isOutput=False
    )
    # weights duplicated across halves: col (b*9 + t)*F + f
    # fp8e4m3: the fake-quant integers in [-7,7] are exact in fp8, and fp8
    # weights halve the LDWEIGHTS time via fast-weight-load (4 elems/read)
    fp8 = mybir.dt.float8e4
    wts = nc.declare_dram_parameter("wts", [128, SW * 9 * F], fp8, isOutput=False)
    bsc_p = nc.declare_dram_parameter("bsc", [128, 2], f32, isOutput=False)
    # b-major output layout: block (b, hh) of an image is contiguous, so
    # each PSUM eviction can stream straight to DRAM. Host reassembles.
    y = nc.declare_dram_parameter(
        "y", [IMGS_PER_CORE, 128, SW, H * WB], bf16, isOutput=True
    )

    WCH = 9 * F                  # weight columns per b-block

    N_WGRP = 6                   # warmup groups of 4 N=256 matmuls each

    with tile.TileContext(nc) as tc:
        with (
            tc.tile_pool(name="consts", bufs=1) as consts,
            tc.tile_pool(name="strips", bufs=4) as strips,
            tc.tile_pool(name="stage", bufs=3) as stage,
            tc.tile_pool(name="psum", bufs=3, space="PSUM") as psum,
            tc.tile_pool(name="wpsum", bufs=2, space="PSUM") as wpsum,
        ):
            # PE warmup with no DMA dependency: accumulation groups of 4
            # back-to-back matmuls, alternating between two PSUM banks so
            # one bank drains while the other streams -> near-100% PE duty,
            # which is what the HAM busy-detector needs to un-throttle.
            warm_src = consts.tile([64, 256], bf16)
            nc.vector.memset(warm_src[:], 0.0)
            for _ in range(N_WGRP):
                wps = wpsum.tile([64, 256], f32, tag="warm")
                for k in range(4):
                    nc.tensor.matmul(
                        wps[:], warm_src[0:64, 0:64], warm_src[0:64, 0:256],
                        start=(k == 0), stop=(k == 3),
                    )

            wt = consts.tile([128, SW * 9 * F], fp8)
            bsc_t = consts.tile([128, 2], f32)
            # weights on the ACT HWDGE ring (per-b chunks), strips on SP
            # ring -> the two rings pipeline; b0 weights + bias first so the
            # PE and the first eviction are unblocked earliest
            nc.scalar.dma_start(wt[:, 0:WCH], wts[:, 0:WCH])
            nc.scalar.dma_start(bsc_t[:], bsc_p[:])
            for b in range(1, SW):
                nc.scalar.dma_start(
                    wt[:, b * WCH:(b + 1) * WCH], wts[:, b * WCH:(b + 1) * WCH]
                )

            halves = {}
            for q in range(PAIRS):
                for hh in range(2):
                    sh = strips.tile([128, HR * WP], bf16, tag="strip")
                    nc.sync.dma_start(
                        sh[:], xin[q, :, hh * HH * WP:(hh * HH + HR) * WP]
                    )
                    halves[(q, hh)] = sh.rearrange("p (h w) -> p h w", w=WP)

            for q in range(PAIRS):
                for hh in range(2):
                    sqv = halves[(q, hh)]
                    for b in range(SW):
                        ps_a = psum.tile([128, HH * WB], f32, tag="ps_a")
                        ps_b = psum.tile([128, HH * WB], f32, tag="ps_b")
                        for t in range(9):
                            i, j = t // 3, t % 3
                            first, last = t == 0, t == 8
                            wcol = (b * 9 + t) * F
                            rhs_a = sqv[0:64, i:i + HH,
                                        b * WB + j:b * WB + j + WB]
                            rhs_b = sqv[64:128, i:i + HH,
                                        b * WB + j:b * WB + j + WB]
                            nc.tensor.matmul(
                                ps_a[:], wt[0:64, wcol:wcol + F], rhs_a,
                                start=first, stop=last,
                            )
                            nc.tensor.matmul(
                                ps_b[:], wt[64:128, wcol:wcol + F], rhs_b,
                                start=first, stop=last,
                            )
                        # evict with fused quant-scale + bias into contiguous
                        # bf16 block tiles, then stream each straight out
                        ev_a = stage.tile([128, HH * WB], bf16, tag="ev_a")
                        ev_b = stage.tile([128, HH * WB], bf16, tag="ev_b")
                        nc.scalar.activation(
                            ev_a[:], ps_a[:],
                            mybir.ActivationFunctionType.Identity,
                            bias=bsc_t[:, 0:1], scale=bsc_t[:, 1:2],
                        )
                        nc.vector.tensor_scalar(
                            ev_b[:], ps_b[:], bsc_t[:, 1:2], bsc_t[:, 0:1],
                            mybir.AluOpType.mult, mybir.AluOpType.add,
                        )
                        lo, hi = hh * HH * WB, (hh + 1) * HH * WB
                        nc.sync.dma_start(y[2 * q, :, b, lo:hi], ev_a[:])
                        nc.scalar.dma_start(y[2 * q + 1, :, b, lo:hi], ev_b[:])

    nc.finalize()
    return nc


def _prep_inputs(inputs, kernel, bias):
    import ml_dtypes

    # fake-quant: integer part exact in bf16, scale folded into eviction
    scale = float(np.max(np.abs(kernel)) / 7.0)
    w_int = np.round(kernel[0] / scale).astype(np.float32)  # [SW, 576, F]

    # weight layout: [128 partitions, SW*9*F]; partition p holds channel
    # c = p % 64 (duplicated across halves for the two PE row groups)
    # free index (b*9 + t)*F + f  ->  w_int[b, c*9 + t, f]
    w4 = w_int.reshape(SW, C, 9, F)                    # [b, c, t, f]
    wt_half = np.transpose(w4, (1, 0, 2, 3)).reshape(C, SW * 9 * F)
    wts_host = np.concatenate([wt_half, wt_half], axis=0)  # [128, SW*9*F]
    wts_host = wts_host.astype(ml_dtypes.float8_e4m3)  # ints in [-7,7]: exact

    # input: pad spatially, bf16, pack image pairs into 128 partitions
    xp = np.zeros((N, C, HP, WP), np.float32)
    xp[:, :, 1:-1, 1:-1] = inputs
    xp = xp.astype(ml_dtypes.bfloat16).reshape(N, C, HP * WP)

    bsc_host = np.zeros((128, 2), np.float32)
    bsc_host[:, 0] = np.ascontiguousarray(bias, np.float32)
    bsc_host[:, 1] = scale

    in_maps = []
    for core in range(N_CORES):
        base = core * IMGS_PER_CORE
        xin = np.empty((PAIRS, 128, HP * WP), ml_dtypes.bfloat16)
        for q in range(PAIRS):
            xin[q, 0:64] = xp[base + 2 * q]
            xin[q, 64:128] = xp[base + 2 * q + 1]
        in_maps.append({
            "xin": xin,
            "wts": wts_host,
            "bsc": bsc_host,
        })
    return in_maps


def kernel(inputs, kernel, bias, _trace=False):
    from concourse.bass_utils import run_bass_kernel_spmd

    inputs = np.asarray(inputs)
    kernel = np.asarray(kernel)
    bias = np.asarray(bias)

    if "nc" not in _COMPILED:
        _COMPILED["nc"] = _build_program()
    nc = _COMPILED["nc"]

    in_maps = _prep_inputs(inputs, kernel, bias)
    res = run_bass_kernel_spmd(
        nc, in_maps, list(range(N_CORES)), trace=_trace
    )
    out = np.empty((N, F, H, W), np.float32)
    for core in range(N_CORES):
        base = core * IMGS_PER_CORE
        # y is b-major: [img, f, b, h*WB+wb] -> [img, f, h, b*WB+wb]
        yc = res.results[core]["y"].astype(np.float32)
        yc = yc.reshape(IMGS_PER_CORE, F, SW, H, WB).transpose(0, 1, 3, 2, 4)
        out[base:base + IMGS_PER_CORE] = yc.reshape(IMGS_PER_CORE, F, H, W)
    if _trace:
        return out, res
    return out
